# revision 8
# baseline (speedup 1.0000x reference)
"""Trainium2 Bass kernel for a pre-LN transformer block (B=4, T=2048, C=1024, H=16).

Sharding: 8 NeuronCores, core c handles batch b=c//2, query-token half c%2.
Each core computes K/V over its batch's visible prefix (kv token order is
[context | own]; for half 0 the context slots are zeros and masked off), full
causal attention for its 1024 query tokens, and the MLP for those tokens.
No collectives; the host concatenates the 8 output slices.

Layout: activations are kept feature-major (x^T: channels on partitions,
tokens on the free dim) so every projection is a plain [K=128]x[M=128]
stationary-weight matmul. Softmax runs on scores^T (k-tokens on partitions),
where the reduction over k is done by the attention-value matmul itself via a
shared-ones column block in the V operand ([v_even | ones | v_odd] per head
pair) -- row block 64:128 (or 0:64) of the AV psum is then the softmax
denominator, pre-broadcast. LayerNorm statistics use a full-ones [128,128]
stationary matmul, which yields partition-broadcast sums directly.

All matmuls are bf16 with f32 PSUM accumulation (M=128 always; M<128 and K=1
matmuls are broken on this toolchain). The causal structure inside the own
block is compile-time (fully-masked blocks are skipped; 4 static triangular
mask tiles handle the diagonal), and the context-valid/invalid choice is a
runtime per-partition bias folded into the exp() activation.
"""

import numpy as np
import ml_dtypes
from contextlib import ExitStack

import concourse.bass as bass
import concourse.mybir as mybir
import concourse.tile as tile
import bass_rust
from concourse.vector_clock import ScopedClock

F32 = mybir.dt.float32
BF16 = mybir.dt.bfloat16
AF = mybir.ActivationFunctionType
ALU = mybir.AluOpType

B, T, C, H = 4, 2048, 1024, 16
D = C // H            # 64
P = 128
CH = C // P           # 8 feature chunks
TOWN = 1024           # query tokens per core
TKV = 2048            # kv tokens per core ([context | own])
KC = TKV // P         # 16 kv chunks
QT = 512              # token tile
NQT = TOWN // QT      # 2
FFI = 4 * C           # 4096
FCH = FFI // P        # 32
NEG = -30.0           # additive mask knocking out invalid context
PS_PAIR = 192         # vtok pair stride: [v_even(64) | ones(64) | v_odd(64)]
AV_LAG = 3            # scores->av pipeline lag (exp latency hiding)

# ---------------------------------------------------------------------------
# Tile patch: this walrus build rejects >1 sync wait per instruction. Split
# multi-wait instructions into single-wait EventSemaphore carriers that
# precede them on the same engine queue; same for the tile-exit drain.
# ---------------------------------------------------------------------------
_patched = False


def _split_waits(self, ordered):
    by_num = {h.num: h for h in self.sems.allocated().values()}
    for bb_name, insts in list(ordered.items()):
        new = []
        for inst in insts:
            si = getattr(inst, "sync_info", None)
            if si is not None and len(si.on_wait) > 1:
                waits = list(si.on_wait)
                sem_w = [w for w in waits
                         if w.sync_type == "semaphore" and w.wait_reg is None
                         and w.id in by_num]
                other = [w for w in waits if w not in sem_w]
                if other:
                    if len(other) > 1:
                        raise RuntimeError(
                            f"{inst.name}: non-splittable waits {other}")
                    keep, carriers = other, sem_w
                else:
                    keep, carriers = [sem_w[-1]], sem_w[:-1]
                ups = [(u.id, u.update_value) for u in si.on_update]
                inst.sync_info = None
                for w in keep:
                    bass_rust.wait_op(inst, by_num[w.id], w.wait_value,
                                      "sem-ge", True)
                for uid, uval in ups:
                    bass_rust.then_inc(inst, by_num[uid], uval, True)
                for w in carriers:
                    c = mybir.InstEventSemaphore(
                        name=self.nc.get_next_instruction_name(),
                        ins=[], outs=[])
                    c.engine = inst.engine
                    c.bass_nofuse = True
                    bass_rust.wait_op(c, by_num[w.id], w.wait_value,
                                      "sem-ge", True)
                    new.append(c)
            if si is not None and len(si.on_update) > 1:
                raise RuntimeError(f"{inst.name}: multi-update {si.on_update}")
            new.append(inst)
        ordered[bb_name] = new


def _apply_tile_patch():
    global _patched
    if _patched:
        return
    _orig_lower = tile.TileContext._lower_ordered_insts

    def _patched_lower(self, ordered):
        _split_waits(self, ordered)
        return _orig_lower(self, ordered)

    def _patched_drain_and_barrier(self, tick_clock, wait_clock):
        nc = self.nc
        drain_inst = nc.sync.drain()
        wait_clock.add_sem_waits(
            drain_inst.ins, ScopedClock({None: tick_clock.global_clock}))
        si = drain_inst.ins.sync_info
        waits = list(si.on_wait) if si is not None else []
        if len(waits) > 1:
            drain_inst.ins.sync_info = None
            by_num = {h.num: h for h in self.sems.allocated().values()}
            for w in waits:
                nc.sync.wait_ge(by_num[w.id], w.wait_value)
        nc.all_engine_barrier()
        popped = nc._tile_sem_poison_stack.pop()
        assert popped is self._sem_poison
        nc.clear_and_free_semaphores(list(self.sems.allocated().values()))
        nc.all_engine_barrier()

    tile.TileContext._lower_ordered_insts = _patched_lower
    tile.TileContext._drain_and_barrier = _patched_drain_and_barrier
    _patched = True


# ---------------------------------------------------------------------------
# Bass program
# ---------------------------------------------------------------------------

def _ln_tile(nc, pools, ps, xsl, ones_full, g_col, g_is1, b_col, b_is0, dst):
    """LayerNorm one [C, QT] token tile.

    xsl: list of CH f32 APs [128, QT] (feature chunks of x^T)
    dst: list of CH bf16 APs [128, QT] to write h^T into
    """
    sb1, sb = pools
    psum_s = ps.tile([P, QT], F32, tag="ln_s")
    psum_q = ps.tile([P, QT], F32, tag="ln_q")
    for o in range(CH):
        xb = sb.tile([P, QT], BF16, tag="ln_xb")
        xsq = sb.tile([P, QT], BF16, tag="ln_xsq")
        nc.vector.tensor_copy(xb[:], xsl[o])
        nc.scalar.activation(xsq[:], xsl[o], AF.Square)
        nc.tensor.matmul(psum_s[:], ones_full[:], xb[:],
                         start=(o == 0), stop=(o == CH - 1))
        nc.tensor.matmul(psum_q[:], ones_full[:], xsq[:],
                         start=(o == 0), stop=(o == CH - 1))
    # mu = S/C ; var+eps = Q/C + (eps - mu^2) ; rstd = 1/sqrt(var+eps)
    mu = sb.tile([P, QT], F32, tag="ln_mu")
    nc.scalar.activation(mu[:], psum_s[:], AF.Identity, scale=1.0 / C)
    var = sb1.tile([P, QT], F32, tag="ln_var")
    nc.vector.tensor_tensor(var[:], mu[:], mu[:], ALU.mult)
    nc.vector.tensor_scalar(var[:], var[:], -1.0, 1e-5, ALU.mult, ALU.add)
    nc.vector.scalar_tensor_tensor(var[:], psum_q[:], 1.0 / C, var[:],
                                   ALU.mult, ALU.add)
    std = sb1.tile([P, QT], F32, tag="ln_std")
    nc.scalar.activation(std[:], var[:], AF.Sqrt)
    rstd = sb.tile([P, QT], F32, tag="ln_rstd")
    nc.vector.reciprocal(rstd[:], std[:])
    for o in range(CH):
        tmp = sb1.tile([P, QT], F32, tag="ln_tmp")
        nc.vector.tensor_tensor(tmp[:], xsl[o], mu[:], ALU.subtract)
        g = 1.0 if g_is1 else g_col[:, o:o + 1]
        nc.vector.scalar_tensor_tensor(dst[o], tmp[:], g, rstd[:],
                                       ALU.mult, ALU.mult)
        if not b_is0:
            nc.vector.tensor_scalar(dst[o], dst[o], b_col[:, o:o + 1],
                                    None, ALU.add)


def build_nc(g1_is1, b1_is0, g2_is1, b2_is0):
    nc = bass.Bass()

    xctxT = nc.dram_tensor("xctxT", [C, TOWN], F32, kind="ExternalInput")
    xownT = nc.dram_tensor("xownT", [C, TOWN], F32, kind="ExternalInput")
    wq = nc.dram_tensor("wq", [C, C], BF16, kind="ExternalInput")
    wk = nc.dram_tensor("wk", [C, C], BF16, kind="ExternalInput")
    wv = nc.dram_tensor("wv", [C, C], BF16, kind="ExternalInput")
    wo = nc.dram_tensor("wo", [C, C], BF16, kind="ExternalInput")
    w1 = nc.dram_tensor("w1", [C, FFI], BF16, kind="ExternalInput")
    w2 = nc.dram_tensor("w2", [FFI, C], BF16, kind="ExternalInput")
    g1c = nc.dram_tensor("g1c", [P, CH], F32, kind="ExternalInput")
    b1lc = nc.dram_tensor("b1lc", [P, CH], F32, kind="ExternalInput")
    g2c = nc.dram_tensor("g2c", [P, CH], F32, kind="ExternalInput")
    b2lc = nc.dram_tensor("b2lc", [P, CH], F32, kind="ExternalInput")
    boc = nc.dram_tensor("boc", [P, CH], F32, kind="ExternalInput")
    b1c = nc.dram_tensor("b1c", [P, FCH], F32, kind="ExternalInput")
    b2c = nc.dram_tensor("b2c", [P, CH], F32, kind="ExternalInput")
    betad = nc.dram_tensor("betad", [P, KC], F32, kind="ExternalInput")
    trimaskd = nc.dram_tensor("trimaskd", [P, 4, QT], BF16,
                              kind="ExternalInput")
    yT = nc.dram_tensor("yT", [C, TOWN], F32, kind="ExternalOutput")
    x2d = nc.dram_tensor("x2d", [C, TOWN], F32)      # internal scratch

    xctxr = xctxT.rearrange("(o p) t -> p o t", p=P)
    xownr = xownT.rearrange("(o p) t -> p o t", p=P)
    x2r = x2d.rearrange("(o p) t -> p o t", p=P)
    yr = yT.rearrange("(o p) t -> p o t", p=P)

    with tile.TileContext(nc) as tc, ExitStack() as st:
        cst = st.enter_context(tc.tile_pool(name="cst", bufs=1))
        sb1 = st.enter_context(tc.tile_pool(name="sb1", bufs=1))
        sb2 = st.enter_context(tc.tile_pool(name="sb2", bufs=2))
        wpool = st.enter_context(tc.tile_pool(name="wpool", bufs=3))

        ones_full = cst.tile([P, P], BF16)
        nc.vector.memset(ones_full[:], 1.0)
        g1t = cst.tile([P, CH], F32); nc.sync.dma_start(g1t[:], g1c[:])
        b1lt = cst.tile([P, CH], F32); nc.sync.dma_start(b1lt[:], b1lc[:])
        g2t = cst.tile([P, CH], F32); nc.sync.dma_start(g2t[:], g2c[:])
        b2lt = cst.tile([P, CH], F32); nc.sync.dma_start(b2lt[:], b2lc[:])
        beta = cst.tile([P, KC], F32); nc.sync.dma_start(beta[:], betad[:])
        trimask = cst.tile([P, 4, QT], BF16)
        nc.sync.dma_start(trimask[:], trimaskd[:])
        bot = cst.tile([P, CH], F32); nc.sync.dma_start(bot[:], boc[:])
        b1t = cst.tile([P, FCH], F32); nc.sync.dma_start(b1t[:], b1c[:])
        b2t = cst.tile([P, CH], F32); nc.sync.dma_start(b2t[:], b2c[:])

        ln_pools = (sb1, sb2)

        with ExitStack() as stA:            # attnfm: alive phases 2..Wo
            atp = stA.enter_context(tc.tile_pool(name="atp", bufs=1))
            attnfm = atp.tile([P, CH, TOWN], BF16)

            with ExitStack() as st12:       # kfm/qfm/vtok: phases 1..2
                akq = st12.enter_context(tc.tile_pool(name="akq", bufs=1))
                kfm = akq.tile([P, CH, TKV], BF16)
                # q zero-padded per head: rows [0:64) hold even heads' q,
                # rows [64:128) odd heads'; the complement is zero so a full
                # K=128 matmul against the packed k chunk contracts exactly
                # one head. (K<128 at base 64 faults this toolchain.)
                qpad = akq.tile([P, H, TOWN], BF16)
                nc.vector.memset(qpad[:], 0.0)
                vtok = akq.tile([P, KC, 8 * PS_PAIR], BF16)
                for j in range(8):
                    nc.vector.memset(
                        vtok[:, :, j * PS_PAIR + D:j * PS_PAIR + 2 * D], 1.0)

                # ---------- phase 1: LN1 + Q/K/V projections ---------------
                with ExitStack() as st1:
                    xs = st1.enter_context(tc.tile_pool(name="xs", bufs=1))
                    hs = st1.enter_context(tc.tile_pool(name="hs", bufs=1))
                    wv_p = st1.enter_context(
                        tc.tile_pool(name="wv_p", bufs=1))
                    ps_proj = st1.enter_context(
                        tc.tile_pool(name="ps_proj1", bufs=3, space="PSUM"))
                    ps_ln = st1.enter_context(
                        tc.tile_pool(name="ps_ln1", bufs=1, space="PSUM"))
                    for tt in range(TKV // QT):       # 0,1 ctx; 2,3 own
                        is_own = tt >= 2
                        xr = xownr if is_own else xctxr
                        t0 = (tt % 2) * QT
                        xt = xs.tile([P, CH, QT], F32, tag="xt")
                        nc.sync.dma_start(xt[:], xr[:, :, t0:t0 + QT])
                        hT = hs.tile([P, CH, QT], BF16, tag="hT")
                        _ln_tile(nc, ln_pools, ps_ln,
                                 [xt[:, o] for o in range(CH)], ones_full,
                                 g1t, g1_is1, b1lt, b1_is0,
                                 [hT[:, o] for o in range(CH)])
                        for co in range(CH):
                            wt = wpool.tile([P, CH, P], BF16, tag="wco")
                            nc.sync.dma_start(
                                wt[:], wk[:, co * P:(co + 1) * P]
                                .rearrange("(o p) n -> p o n", p=P))
                            pt = ps_proj.tile([P, QT], F32, tag="proj")
                            for o in range(CH):
                                nc.tensor.matmul(pt[:], wt[:, o], hT[:, o],
                                                 start=(o == 0),
                                                 stop=(o == CH - 1))
                            nc.scalar.copy(
                                kfm[:, co, tt * QT:(tt + 1) * QT], pt[:])
                        for nt in range(2):
                            wt = wv_p.tile([P, CH, QT], BF16, tag="wnt")
                            nc.sync.dma_start(
                                wt[:], wv[:, nt * QT:(nt + 1) * QT]
                                .rearrange("(o p) n -> p o n", p=P))
                            for tci in range(4):
                                tc_ = tt * 4 + tci
                                pt = ps_proj.tile([P, QT], F32, tag="proj")
                                for o in range(CH):
                                    nc.tensor.matmul(
                                        pt[:],
                                        hT[:, o, tci * P:(tci + 1) * P],
                                        wt[:, o], start=(o == 0),
                                        stop=(o == CH - 1))
                                pr = pt[:].rearrange(
                                    "p (j hd) -> p j hd", hd=2 * D)
                                dst = vtok[:, tc_, nt * 4 * PS_PAIR:
                                           (nt + 1) * 4 * PS_PAIR] \
                                    .rearrange("p (j s) -> p j s", s=PS_PAIR)
                                nc.vector.tensor_copy(dst[:, :, 0:D],
                                                      pr[:, :, 0:D])
                                nc.vector.tensor_copy(dst[:, :, 2 * D:3 * D],
                                                      pr[:, :, D:2 * D])
                        if is_own:
                            qt0 = (tt - 2) * QT
                            for co in range(CH):
                                wt = wpool.tile([P, CH, P], BF16, tag="wco")
                                nc.sync.dma_start(
                                    wt[:], wq[:, co * P:(co + 1) * P]
                                    .rearrange("(o p) n -> p o n", p=P))
                                pt = ps_proj.tile([P, QT], F32, tag="proj")
                                for o in range(CH):
                                    nc.tensor.matmul(
                                        pt[:], wt[:, o], hT[:, o],
                                        start=(o == 0), stop=(o == CH - 1))
                                nc.scalar.copy(
                                    qpad[0:D, 2 * co, qt0:qt0 + QT],
                                    pt[0:D, :])
                                nc.scalar.copy(
                                    qpad[D:P, 2 * co + 1, qt0:qt0 + QT],
                                    pt[D:P, :])

                # ---------- phase 2: attention -----------------------------
                with tc.tile_pool(name="ps_sc", bufs=3,
                                  space="PSUM") as ps_sc, \
                     tc.tile_pool(name="ps_av", bufs=2,
                                  space="PSUM") as ps_av, \
                     tc.tile_pool(name="esb", bufs=AV_LAG + 3) as esb, \
                     tc.tile_pool(name="lsb", bufs=2) as lsb:
                    for qt in range(NQT):
                        for h in range(H):
                            co, hi = h // 2, h % 2
                            base = hi * D
                            ksl = kfm[:, co]
                            qsl = qpad[:, h, qt * QT:(qt + 1) * QT]
                            chunks = list(range(8)) + \
                                [8 + i for i in range(4 * (qt + 1))]
                            vbase = co * PS_PAIR + (0 if hi == 0 else D)
                            av = ps_av.tile([P, QT], F32, tag="av")
                            exps = []

                            def av_mm(i, av=av, chunks=chunks, exps=exps,
                                      vbase=vbase):
                                kc_i, e = exps[i]
                                nc.tensor.matmul(
                                    av[:], vtok[:, kc_i, vbase:vbase + P],
                                    e[:], start=(i == 0),
                                    stop=(i == len(chunks) - 1))

                            for i, kc_i in enumerate(chunks):
                                sc = ps_sc.tile([P, QT], F32, tag="sc")
                                nc.tensor.matmul(
                                    sc[:], ksl[:, kc_i * P:(kc_i + 1) * P],
                                    qsl, start=True, stop=True)
                                e = esb.tile([P, QT], BF16, tag="e")
                                nc.scalar.activation(
                                    e[:], sc[:], AF.Exp, scale=0.125,
                                    bias=beta[:, kc_i:kc_i + 1])
                                own_loc = kc_i - 8
                                if kc_i >= 8 and own_loc >= 4 * qt:
                                    nc.vector.tensor_tensor(
                                        e[:], e[:],
                                        trimask[:, own_loc - 4 * qt],
                                        ALU.mult)
                                exps.append((kc_i, e))
                                if i >= AV_LAG:
                                    av_mm(i - AV_LAG)
                            for i in range(max(0, len(chunks) - AV_LAG),
                                           len(chunks)):
                                av_mm(i)
                            # even head ([v|ones]): rows 0:64 av, 64:128 l;
                            # odd head ([ones|v]): rows 0:64 l, 64:128 av
                            arow, lrow = (0, D) if hi == 0 else (D, 0)
                            linv = lsb.tile([D, QT], F32, tag="linv")
                            nc.vector.reciprocal(linv[:],
                                                 av[lrow:lrow + D, :])
                            nc.vector.tensor_tensor(
                                attnfm[base:base + D, co,
                                       qt * QT:(qt + 1) * QT],
                                av[arow:arow + D, :], linv[:], ALU.mult)

            # ---------- phase 3a: Wo + residual -> x2d (DRAM) --------------
            with tc.tile_pool(name="xo_p", bufs=2) as xo_p, \
                 tc.tile_pool(name="ps_proj3", bufs=3, space="PSUM") \
                 as ps_proj:
                for co in range(CH):
                    wt = wpool.tile([P, CH, P], BF16, tag="wco")
                    nc.sync.dma_start(
                        wt[:], wo[:, co * P:(co + 1) * P]
                        .rearrange("(o p) n -> p o n", p=P))
                    for tt in range(NQT):
                        sl = slice(tt * QT, (tt + 1) * QT)
                        xo = xo_p.tile([P, QT], F32, tag="xo")
                        nc.sync.dma_start(xo[:], xownr[:, co, sl])
                        pt = ps_proj.tile([P, QT], F32, tag="proj")
                        for o in range(CH):
                            nc.tensor.matmul(pt[:], wt[:, o],
                                             attnfm[:, o, sl],
                                             start=(o == 0),
                                             stop=(o == CH - 1))
                        tmp = sb1.tile([P, QT], F32, tag="wo_t")
                        nc.vector.tensor_scalar(tmp[:], pt[:],
                                                bot[:, co:co + 1],
                                                None, ALU.add)
                        x2t = xo_p.tile([P, QT], F32, tag="x2t")
                        nc.vector.tensor_tensor(x2t[:], tmp[:], xo[:],
                                                ALU.add)
                        nc.sync.dma_start(x2r[:, co, sl], x2t[:])

        # ---------- phase 3b: LN2 (x2d -> h2 in SBUF) -----------------------
        with ExitStack() as st34:
            h2p = st34.enter_context(tc.tile_pool(name="h2p", bufs=1))
            h2 = h2p.tile([P, CH, TOWN], BF16)
            with tc.tile_pool(name="x2s", bufs=1) as x2s, \
                 tc.tile_pool(name="ps_ln3", bufs=1, space="PSUM") as ps_ln:
                for tt in range(NQT):
                    sl = slice(tt * QT, (tt + 1) * QT)
                    x2t = x2s.tile([P, CH, QT], F32, tag="x2s")
                    nc.sync.dma_start(x2t[:], x2r[:, :, sl])
                    _ln_tile(nc, ln_pools, ps_ln,
                             [x2t[:, o] for o in range(CH)], ones_full,
                             g2t, g2_is1, b2lt, b2_is0,
                             [h2[:, o, sl] for o in range(CH)])

            # ---------- phase 4: FFN ---------------------------------------
            with ExitStack() as st4:
                f1 = st4.enter_context(tc.tile_pool(name="f1", bufs=1))
                w2p = st4.enter_context(tc.tile_pool(name="w2p", bufs=2))
                x2f = st4.enter_context(tc.tile_pool(name="x2f", bufs=2))
                ps_proj = st4.enter_context(
                    tc.tile_pool(name="ps_proj4", bufs=3, space="PSUM"))
                ffn1 = f1.tile([P, FCH, TOWN], BF16)
                for cm in range(FCH):
                    wt = wpool.tile([P, CH, P], BF16, tag="wco")
                    nc.sync.dma_start(
                        wt[:], w1[:, cm * P:(cm + 1) * P]
                        .rearrange("(o p) n -> p o n", p=P))
                    for tt in range(NQT):
                        pt = ps_proj.tile([P, QT], F32, tag="proj")
                        for o in range(CH):
                            nc.tensor.matmul(
                                pt[:], wt[:, o],
                                h2[:, o, tt * QT:(tt + 1) * QT],
                                start=(o == 0), stop=(o == CH - 1))
                        nc.scalar.activation(
                            ffn1[:, cm, tt * QT:(tt + 1) * QT], pt[:],
                            AF.Relu, bias=b1t[:, cm:cm + 1])
                for co in range(CH):
                    wt = w2p.tile([P, FCH, P], BF16, tag="w2co")
                    nc.sync.dma_start(
                        wt[:], w2[:, co * P:(co + 1) * P]
                        .rearrange("(o p) n -> p o n", p=P))
                    for tt in range(NQT):
                        sl = slice(tt * QT, (tt + 1) * QT)
                        x2t = x2f.tile([P, QT], F32, tag="x2f")
                        nc.sync.dma_start(x2t[:], x2r[:, co, sl])
                        pt = ps_proj.tile([P, QT], F32, tag="proj")
                        for o in range(FCH):
                            nc.tensor.matmul(pt[:], wt[:, o],
                                             ffn1[:, o, sl],
                                             start=(o == 0),
                                             stop=(o == FCH - 1))
                        ytile = sb1.tile([P, QT], F32, tag="y_t")
                        nc.vector.tensor_scalar(ytile[:], pt[:],
                                                b2t[:, co:co + 1],
                                                None, ALU.add)
                        nc.vector.tensor_tensor(ytile[:], ytile[:],
                                                x2t[:], ALU.add)
                        nc.sync.dma_start(yr[:, co, sl], ytile[:])
    return nc


# ---------------------------------------------------------------------------
# Host wrapper
# ---------------------------------------------------------------------------

def _col_layout(v, chunks):
    return np.ascontiguousarray(np.asarray(v, np.float32).reshape(chunks, P).T)


_CACHE = {}


def _prepare(inputs):
    x = np.asarray(inputs["x"], np.float32)
    key = (bool(np.all(np.asarray(inputs["ln1_g"]) == 1)),
           bool(np.all(np.asarray(inputs["ln1_b"]) == 0)),
           bool(np.all(np.asarray(inputs["ln2_g"]) == 1)),
           bool(np.all(np.asarray(inputs["ln2_b"]) == 0)))

    bf = ml_dtypes.bfloat16
    shared = {
        "wq": np.asarray(inputs["Wq"], np.float32).astype(bf),
        "wk": np.asarray(inputs["Wk"], np.float32).astype(bf),
        "wv": np.asarray(inputs["Wv"], np.float32).astype(bf),
        "wo": np.asarray(inputs["Wo"], np.float32).astype(bf),
        "w1": np.asarray(inputs["W1"], np.float32).astype(bf),
        "w2": np.asarray(inputs["W2"], np.float32).astype(bf),
        "g1c": _col_layout(inputs["ln1_g"], CH),
        "b1lc": _col_layout(inputs["ln1_b"], CH),
        "g2c": _col_layout(inputs["ln2_g"], CH),
        "b2lc": _col_layout(inputs["ln2_b"], CH),
        "boc": _col_layout(inputs["bo"], CH),
        "b1c": _col_layout(inputs["b1"], FCH),
        "b2c": _col_layout(inputs["b2"], CH),
    }
    tri = np.zeros((P, 4, QT), np.float32)
    ii = np.arange(QT)[None, :]
    kk = np.arange(P)[:, None]
    for r in range(4):
        tri[:, r, :] = (ii >= r * P + kk).astype(np.float32)
    shared["trimaskd"] = tri.astype(bf)

    in_maps = []
    for core in range(8):
        b, half = core // 2, core % 2
        xT = np.ascontiguousarray(x[b].T)          # [C, T]
        own = np.ascontiguousarray(xT[:, half * TOWN:(half + 1) * TOWN])
        ctx = (np.ascontiguousarray(xT[:, 0:TOWN]) if half == 1
               else np.zeros((C, TOWN), np.float32))
        beta = np.zeros((P, KC), np.float32)
        if half == 0:
            beta[:, 0:8] = NEG
        m = dict(shared)
        m["xownT"] = own
        m["xctxT"] = ctx
        m["betad"] = beta
        in_maps.append(m)
    return key, in_maps


def kernel(**inputs):
    _apply_tile_patch()
    from concourse.bass_utils import run_bass_kernel_spmd

    key, in_maps = _prepare(inputs)
    if key not in _CACHE:
        _CACHE[key] = build_nc(*key)
    nc = _CACHE[key]

    res = run_bass_kernel_spmd(nc, in_maps, core_ids=list(range(8)))
    y = np.empty((B, T, C), np.float32)
    for core in range(8):
        b, half = core // 2, core % 2
        y[b, half * TOWN:(half + 1) * TOWN, :] = res.results[core]["yT"].T
    return y
